# revision 36
# baseline (speedup 1.0000x reference)
"""MLA (multi-head latent attention) forward on 8 TRN2 NeuronCores.

Sharding: 2-way data-parallel over batch x 4-way tensor-parallel over heads.
Core c handles batch b=c//4 and heads 4g..4g+3 where g=c%4. Each core runs an
identical SPMD program on its shard; the host sums the 4 partial outputs per
batch (the o_proj contribution of each head group) and transposes.

On-chip layout is feature-major ([feature, token]) so every matmul contracts
over the partition dim without extra transposes; only the input x is
transposed once on the TensorEngine. Probabilities are computed transposed
(s[tk, tq]) so softmax needs no max-subtraction (scores are bounded ~6) and
P@V contracts naturally; denominators come from ones-matmuls. RMSNorm scaling
commutes with the B-projections (it is per-token), so the q-path scale is
applied to the q_b outputs, keeping the norm off the TensorE critical path.
"""

import numpy as np
import ml_dtypes
from contextlib import ExitStack

B, T, HIDDEN = 2, 2048, 2048
NUM_HEADS = 16
QK_NOPE, QK_ROPE, HEAD_DIM, V_HEAD = 128, 64, 192, 128
KV_LORA, Q_LORA = 512, 1536
EPS = 1e-6
NCORES = 8
HPC = 4  # heads per core

BF16 = ml_dtypes.bfloat16
NEG = -1e9

_CACHE = {}


def _build():
    import concourse.bass as bass
    import concourse.tile as tile
    from concourse import bacc, mybir
    from concourse.bass import ts

    f32 = mybir.dt.float32
    bf = mybir.dt.bfloat16
    AF = mybir.ActivationFunctionType

    nc = bacc.Bacc(
        "TRN2",
        target_bir_lowering=False,
        debug=False,
        enable_asserts=True,
        num_devices=NCORES,
    )

    def din(name, shape, dt=bf):
        return nc.dram_tensor(name, shape, dt, kind="ExternalInput").ap()

    x_ap = din("x", [T, HIDDEN])                    # [t, d]
    qaw_ap = din("qaw", [HIDDEN, Q_LORA])           # [d, lat]
    # qbw columns reordered on host per head-pair: [nope0|nope1|rope0|rope1]
    qbw_ap = din("qbw", [Q_LORA, HPC * HEAD_DIM])   # ln + 1/sqrt(dh) folded
    kvaw_ap = din("kvaw", [HIDDEN, KV_LORA + HPC * QK_ROPE])
    kvbw_ap = din("kvbw", [KV_LORA, HPC * (QK_NOPE + V_HEAD)])  # ln folded
    ow_ap = din("ow", [HPC * V_HEAD, HIDDEN])
    mask_ap = din("mask", [128, 896])               # 0/1 causal mask bank (bf16)
    ident_ap = din("ident", [128, 128])
    ones128_ap = din("ones128", [128, 1])           # bf16, den lhsT
    out_ap = nc.dram_tensor("out", [HIDDEN, T], bf, kind="ExternalOutput").ap()

    KC = HIDDEN // 128   # 16 contraction chunks over hidden
    TT = T // 128        # 16 token tiles of 128
    TQ = T // 512        # 4 token chunks of 512
    NQ = Q_LORA // 128   # 12 q-latent col tiles
    NKV = (KV_LORA + HPC * QK_ROPE) // 128  # 6 kv_a col tiles (4 latent + 2 rope)
    NL = KV_LORA // 128  # 4 latent tiles

    def eng(idx):
        return nc.scalar if idx % 2 else nc.vector

    def copy(e, out, in_):
        if e is nc.scalar:
            nc.scalar.copy(out, in_)
        else:
            nc.vector.tensor_copy(out, in_)

    with tile.TileContext(nc) as tc:
        with tc.tile_pool(name="consts", bufs=1) as consts, \
             tc.tile_pool(name="trans", bufs=3) as trans, \
             tc.tile_pool(name="dram", bufs=1, space="DRAM") as dram, \
             tc.tile_pool(name="wD", bufs=1) as pw, \
             tc.tile_pool(name="owp", bufs=1) as powp, \
             tc.tile_pool(name="act", bufs=1) as act:

            mask = consts.tile([128, 896], bf)
            nc.sync.dma_start(out=mask, in_=mask_ap)
            ones128 = consts.tile([128, 1], bf)
            nc.sync.dma_start(out=ones128, in_=ones128_ap)
            eps1 = consts.tile([1, 1], f32)
            nc.vector.memset(eps1, EPS)

            # activations, feature-major; kv latent normalized in place
            xq = act.tile([128, NQ, T], bf)
            xkv = act.tile([128, NKV, T], bf)
            rq_b = act.tile([128, T], f32)    # broadcast 1/rms for q (per token)
            rkv_b = act.tile([128, T], f32)   # broadcast 1/rms for kv latent
            rkvT = act.tile([128, T // 128], f32)  # rstd_kv as per-partition cols

            # ---- Stage A, t-chunk-major: per 512-token chunk, transpose x
            # strips via the DMA XBAR, then run all A-projection columns for
            # that chunk. Weights re-stream per chunk on the (otherwise idle)
            # gpsimd queue; the small xT working set frees SBUF so the D/F
            # weights (qbw/kvbw/ow) preload during stage A on the sync queue.
            qbws = []
            with tc.tile_pool(name="stageA", bufs=1) as pA, \
                 tc.tile_pool(name="wa", bufs=4) as pwa, \
                 tc.tile_pool(name="pB", bufs=1) as pB, \
                 tc.tile_pool(name="psumA", bufs=1, space="PSUM") as psumA:
                for tci in range(TQ):
                    xTc = pA.tile([128, KC, 512], bf, tag="xTc", bufs=2)
                    for k in range(KC):
                        nc.sync.dma_start(
                            out=xTc[:, k, :],
                            in_=x_ap[ts(tci, 512), ts(k, 128)],
                            transpose=True,
                        )
                    if tci == 0:
                        for pair in range(2):
                            qbw = pw.tile([128, NQ, 2 * HEAD_DIM], bf, tag="qbw",
                                          bufs=2, name=f"qbw{pair}")
                            nc.sync.dma_start(
                                out=qbw,
                                in_=qbw_ap[:, ts(pair, 2 * HEAD_DIM)].rearrange(
                                    "(kk p) n -> p kk n", p=128
                                ),
                            )
                            qbws.append(qbw)
                        kvbw = pw.tile([128, NL, HPC * (QK_NOPE + V_HEAD)], bf)
                        nc.sync.dma_start(
                            out=kvbw,
                            in_=kvbw_ap.rearrange("(kk p) n -> p kk n", p=128),
                        )
                        ow = powp.tile([128, HPC, HIDDEN], bf)
                        nc.sync.dma_start(
                            out=ow, in_=ow_ap.rearrange("(kk p) n -> p kk n", p=128)
                        )
                    for src_ap, ncols, dst, tagn in (
                        (qaw_ap, NQ, xq, "wa"), (kvaw_ap, NKV, xkv, "wa")
                    ):
                        for n in range(ncols):
                            wa = pwa.tile([128, KC, 128], bf, tag=tagn, bufs=3)
                            nc.gpsimd.dma_start(
                                out=wa,
                                in_=src_ap[:, ts(n, 128)].rearrange(
                                    "(kk p) n -> p kk n", p=128
                                ),
                            )
                            psm = psumA.tile([128, 512], f32, tag="psm", bufs=3)
                            for kk in range(KC):
                                nc.tensor.matmul(
                                    out=psm,
                                    lhsT=wa[:, kk, :],
                                    rhs=xTc[:, kk, :],
                                    start=(kk == 0),
                                    stop=(kk == KC - 1),
                                )
                            copy(eng(n + tci), dst[:, n, ts(tci, 512)], psm)

                # ---- Stage B: rstd = 1/sqrt(mean(x^2)+eps), then broadcast
                # across partitions with a stride-0 DMA. Squares on DVE (4x
                # bf16 mode); per-token sum over features via ones-matmul.
                rstd_q = pB.tile([1, T], f32, tag="rstd_q", bufs=1)
                rstd_kv = pB.tile([1, T], f32, tag="rstd_kv", bufs=1)
                for t in range(TQ):
                    for src, nn, lora, rstd in (
                        (xq, NQ, Q_LORA, rstd_q),
                        (xkv, NL, KV_LORA, rstd_kv),
                    ):
                        psd = psumA.tile([1, 512], f32, tag="psd", bufs=2)
                        for n in range(nn):
                            sq = pB.tile([128, 512], bf, tag="sq", bufs=3)
                            nc.vector.tensor_mul(
                                sq, src[:, n, ts(t, 512)], src[:, n, ts(t, 512)]
                            )
                            nc.tensor.matmul(
                                out=psd,
                                lhsT=ones128,
                                rhs=sq,
                                start=(n == 0),
                                stop=(n == nn - 1),
                            )
                        tmp = pB.tile([1, 512], f32, tag="tmp", bufs=1)
                        nc.scalar.activation(
                            out=tmp, in_=psd, func=AF.Sqrt, bias=eps1,
                            scale=1.0 / lora,
                        )
                        nc.vector.reciprocal_approx_fast(
                            out=rstd[:, ts(t, 512)], in_=tmp
                        )
                # partition-broadcast must source from DRAM; bounce through it
                rstdq_d = dram.tile([1, T], f32)
                rkv_d = dram.tile([1, T], f32)
                nc.gpsimd.dma_start(out=rstdq_d, in_=rstd_q)
                nc.gpsimd.dma_start(out=rkv_d, in_=rstd_kv)
                nc.gpsimd.dma_start(out=rq_b, in_=rstdq_d.to_broadcast((128, T)))
                nc.gpsimd.dma_start(out=rkv_b, in_=rkv_d.to_broadcast((128, T)))
                nc.gpsimd.dma_start(
                    out=rkvT,
                    in_=rkv_d.rearrange("o (tt p) -> (o p) tt", p=128),
                )

            # kv latent stays RAW: rstd_kv commutes past kv_b, so kn columns
            # and v rows are scaled at copy-out instead (no matmul waits on it)
            kv_lat = xkv

            # ---- Stages D/E/F
            with tc.tile_pool(name="att", bufs=1) as patt, \
                 tc.tile_pool(name="psumD", bufs=1, space="PSUM") as psumD:
                qn = patt.tile([128, HPC, T], bf)
                qr = patt.tile([128, 2, T], bf)   # head pairs packed 64|64
                kn = patt.tile([128, HPC, T], bf)
                vv = patt.tile([128, TT, HPC * V_HEAD], bf)

                # ---- Stage D: q (scaled by rstd_q on copy-out), k_nope, v
                if True:
                    for pair in range(2):
                        qbw = qbws[pair]
                        for t in range(TQ):
                            # cols: [nope0 | nope1 | rope0+rope1]
                            for sub in range(3):
                                ps = psumD.tile([128, 512], f32, tag="psm", bufs=2)
                                for kk in range(NQ):
                                    nc.tensor.matmul(
                                        out=ps,
                                        lhsT=qbw[:, kk, ts(sub, 128)],
                                        rhs=xq[:, kk, ts(t, 512)],
                                        start=(kk == 0),
                                        stop=(kk == NQ - 1),
                                    )
                                dstv = (qn[:, 2 * pair, ts(t, 512)],
                                        qn[:, 2 * pair + 1, ts(t, 512)],
                                        qr[:, pair, ts(t, 512)])[sub]
                                nc.vector.tensor_mul(dstv, ps, rq_b[:, ts(t, 512)])
                    for h in range(HPC):
                        for t in range(TQ):
                            ps3 = psumD.tile([128, 512], f32, tag="psm", bufs=2)
                            for kk in range(NL):
                                nc.tensor.matmul(
                                    out=ps3,
                                    lhsT=kvbw[:, kk, ts(h, 256)][:, 0:128],
                                    rhs=kv_lat[:, kk, ts(t, 512)],
                                    start=(kk == 0),
                                    stop=(kk == NL - 1),
                                )
                            nc.vector.tensor_mul(
                                kn[:, h, ts(t, 512)], ps3, rkv_b[:, ts(t, 512)]
                            )
                    vcols = kvbw.rearrange(
                        "p kk (h two dv) -> p kk h two dv", h=HPC, two=2
                    )
                    for tt in range(TT):
                        psv = psumD.tile([128, 512], f32, tag="psm", bufs=2)
                        for kk in range(NL):
                            nc.tensor.matmul(
                                out=psv,
                                lhsT=kv_lat[:, kk, ts(tt, 128)],
                                rhs=vcols[:, kk, :, 1, :],
                                start=(kk == 0),
                                stop=(kk == NL - 1),
                            )
                        nc.scalar.mul(vv[:, tt, :], psv, mul=rkvT[:, tt:tt + 1])

                # ---- Stage E+F: causal attention (transposed probs) per query
                # chunk; o_proj runs one chunk behind so the softmax tail of
                # chunk i hides under chunk i+1's score matmuls.
                with tc.tile_pool(name="attn_i", bufs=2) as pai, \
                     tc.tile_pool(name="ob", bufs=3) as pob, \
                     tc.tile_pool(name="rdb", bufs=3) as prdb:
                    attn_tiles = []

                    def attention_chunk(i):
                        attn_i = pai.tile([128, HPC, 512], bf, tag="attn_i", bufs=2)
                        for h in range(HPC):
                            nj = 4 * i + 4
                            pso = psumD.tile([128, 512], f32, tag="pso", bufs=2)
                            psd = psumD.tile([1, 512], f32, tag="psd", bufs=1)
                            hp = 64 * (h % 2)

                            def consume(jc, ex, first, last):
                                nc.tensor.matmul(
                                    out=psd, lhsT=ones128, rhs=ex,
                                    start=first, stop=last,
                                )
                                nc.tensor.matmul(
                                    out=pso, lhsT=vv[:, jc, ts(h, V_HEAD)], rhs=ex,
                                    start=first, stop=last,
                                )

                            pending = []
                            for j in range(nj):
                                pss = psumD.tile([128, 512], f32, tag="pss", bufs=3)
                                nc.tensor.matmul(
                                    out=pss,
                                    lhsT=kn[:, h, ts(j, 128)],
                                    rhs=qn[:, h, ts(i, 512)],
                                    start=True,
                                    stop=False,
                                )
                                nc.tensor.matmul(
                                    out=pss,
                                    lhsT=xkv[hp:hp + 64, NL + h // 2, ts(j, 128)],
                                    rhs=qr[hp:hp + 64, h // 2, ts(i, 512)],
                                    start=False,
                                    stop=True,
                                )
                                while len(pending) > 1:
                                    jc, exc = pending.pop(0)
                                    consume(jc, exc, jc == 0, False)
                                ex = trans.tile([128, 512], bf, tag="ex", bufs=3)
                                nc.scalar.activation(out=ex, in_=pss, func=AF.Exp)
                                off = j * 128 - i * 512
                                if off >= 0:
                                    nc.vector.tensor_mul(
                                        ex, ex, mask[:, 384 - off:896 - off]
                                    )
                                pending.append((j, ex))
                            for jc, exc in pending:
                                consume(jc, exc, jc == 0, jc == nj - 1)
                            rd = trans.tile([1, 512], f32, tag="rd", bufs=1)
                            nc.vector.reciprocal_approx_fast(out=rd, in_=psd)
                            rd_bf = trans.tile([1, 512], bf, tag="rd_bf", bufs=2)
                            nc.scalar.copy(rd_bf, rd)
                            rd_d = dram.tile([1, 512], bf, tag="rd_d", bufs=3)
                            nc.gpsimd.dma_start(out=rd_d, in_=rd_bf)
                            rdb = prdb.tile([128, 512], bf, tag="rdb", bufs=2)
                            nc.sync.dma_start(out=rdb, in_=rd_d.to_broadcast((128, 512)))
                            nc.vector.tensor_mul(attn_i[:, h, :], pso, rdb)
                        attn_tiles.append(attn_i)

                    def oproj_chunk(i):
                        attn_i = attn_tiles[i]
                        for m in range(TT):
                            psf = psumD.tile([128, 512], f32, tag="psm", bufs=2)
                            for kk in range(HPC):
                                nc.tensor.matmul(
                                    out=psf,
                                    lhsT=ow[:, kk, ts(m, 128)],
                                    rhs=attn_i[:, kk, :],
                                    start=(kk == 0),
                                    stop=(kk == HPC - 1),
                                )
                            ob = pob.tile([128, 512], bf, tag="ob", bufs=2)
                            copy(eng(m), ob, psf)
                            nc.gpsimd.dma_start(
                                out=out_ap[ts(m, 128), ts(i, 512)], in_=ob
                            )

                    attention_chunk(0)
                    for i in range(1, TQ):
                        attention_chunk(i)
                        oproj_chunk(i - 1)
                    oproj_chunk(TQ - 1)

    nc.compile()
    return nc


def _prep(inputs):
    x = np.asarray(inputs["hidden_states"], np.float32)
    qaw = np.asarray(inputs["q_a_w"], np.float32)
    qalw = np.asarray(inputs["q_a_ln_w"], np.float32)
    qbw = np.asarray(inputs["q_b_w"], np.float32)
    kvaw = np.asarray(inputs["kv_a_w"], np.float32)
    kvlw = np.asarray(inputs["kv_a_ln_w"], np.float32)
    kvbw = np.asarray(inputs["kv_b_w"], np.float32)
    ow = np.asarray(inputs["o_w"], np.float32)

    scale = 1.0 / np.sqrt(np.float32(HEAD_DIM))
    qbw_f = (qbw * qalw[:, None] * scale).astype(BF16)
    kvbw_f = (kvbw * kvlw[:, None]).astype(BF16)
    qaw_b = qaw.astype(BF16)

    r = np.arange(128)[:, None]
    j = np.arange(896)[None, :]
    mask = np.where((j - 384) >= r, 1.0, 0.0).astype(BF16)
    ident = np.eye(128, dtype=BF16)
    ones128 = np.ones((128, 1), BF16)

    in_maps = []
    for c in range(NCORES):
        b, g = c // 4, c % 4
        # group slice of q_b, then reorder per head pair: nope0|nope1|rope0|rope1
        qbw_g = qbw_f[:, g * HPC * HEAD_DIM:(g + 1) * HPC * HEAD_DIM]
        cols = []
        for pair in range(HPC // 2):
            h0, h1 = 2 * pair, 2 * pair + 1
            cols.append(qbw_g[:, h0 * HEAD_DIM:h0 * HEAD_DIM + QK_NOPE])
            cols.append(qbw_g[:, h1 * HEAD_DIM:h1 * HEAD_DIM + QK_NOPE])
            cols.append(qbw_g[:, h0 * HEAD_DIM + QK_NOPE:(h0 + 1) * HEAD_DIM])
            cols.append(qbw_g[:, h1 * HEAD_DIM + QK_NOPE:(h1 + 1) * HEAD_DIM])
        qbw_c = np.ascontiguousarray(np.concatenate(cols, axis=1))

        in_maps.append({
            "x": x[b].astype(BF16),
            "qaw": qaw_b,
            "qbw": qbw_c,
            "kvaw": np.ascontiguousarray(np.concatenate(
                [kvaw[:, :KV_LORA],
                 kvaw[:, KV_LORA + g * HPC * QK_ROPE:
                      KV_LORA + (g + 1) * HPC * QK_ROPE]],
                axis=1).astype(BF16)),
            "kvbw": np.ascontiguousarray(kvbw_f[:, g * HPC * 256:(g + 1) * HPC * 256]),
            "ow": np.ascontiguousarray(
                ow[g * HPC * V_HEAD:(g + 1) * HPC * V_HEAD].astype(BF16)),
            "mask": mask,
            "ident": ident,
            "ones128": ones128,
        })
    return in_maps


def kernel(**inputs):
    from concourse.bass_utils import run_bass_kernel_spmd

    if "nc" not in _CACHE:
        _CACHE["nc"] = _build()
    nc = _CACHE["nc"]
    in_maps = _prep(inputs)
    res = run_bass_kernel_spmd(nc, in_maps, core_ids=list(range(NCORES)),
                               **_CACHE.get("run_kwargs", {}))
    _CACHE["last_results"] = res
    out = np.zeros((B, T, HIDDEN), np.float32)
    for c in range(NCORES):
        out[c // 4] += np.asarray(res.results[c]["out"], np.float32).T
    return out


# revision 37
# speedup vs baseline: 1.0977x; 1.0977x over previous
"""MLA (multi-head latent attention) forward on 8 TRN2 NeuronCores.

Sharding: 2-way data-parallel over batch x 4-way tensor-parallel over heads.
Core c handles batch b=c//4 and heads 4g..4g+3 where g=c%4. Each core runs an
identical SPMD program on its shard; the host sums the 4 partial outputs per
batch (the o_proj contribution of each head group) and transposes.

Layout: activations are feature-major ([feature, token]) so every matmul
contracts over the partition dim; x is transposed once by the DMA XBAR.
Probabilities are computed transposed (s[tk, tq]) so softmax needs no
max-subtraction (scores are bounded ~6) and P@V contracts naturally;
denominators come from ones-matmuls + fast reciprocal + DRAM-bounce
partition-broadcast. RMSNorm scaling is per-token so it commutes with the
B-projections: both q and kv normalizations are applied at copy-out of the
projected tensors, keeping the whole norm pipeline off the TensorE stream.
Weights are pre-tiled on the host so every weight DMA is contiguous.
"""

import numpy as np
import ml_dtypes

B, T, HIDDEN = 2, 2048, 2048
NUM_HEADS = 16
QK_NOPE, QK_ROPE, HEAD_DIM, V_HEAD = 128, 64, 192, 128
KV_LORA, Q_LORA = 512, 1536
EPS = 1e-6
NCORES = 8
HPC = 4  # heads per core

KC = HIDDEN // 128
TT = T // 128
TQ = T // 512
NQ = Q_LORA // 128
NKV = (KV_LORA + HPC * QK_ROPE) // 128
NL = KV_LORA // 128

BF16 = ml_dtypes.bfloat16

_CACHE = {}


def _build():
    import concourse.bass as bass
    import concourse.tile as tile
    from concourse import bacc, mybir
    from concourse.bass import ts

    f32 = mybir.dt.float32
    bf = mybir.dt.bfloat16
    AF = mybir.ActivationFunctionType

    nc = bacc.Bacc(
        "TRN2",
        target_bir_lowering=False,
        debug=False,
        enable_asserts=True,
        num_devices=NCORES,
    )

    def din(name, shape, dt=bf):
        return nc.dram_tensor(name, shape, dt, kind="ExternalInput").ap()

    # weights pre-tiled on host: contiguous per-tile DMA loads
    x_ap = din("x", [T, HIDDEN])                      # [t, d] (XBAR-transposed)
    qaw_ap = din("qaw", [NQ, 128, KC, 128])           # per col-block [p, kk, c]
    kvaw_ap = din("kvaw", [NKV, 128, KC, 128])
    qbw_ap = din("qbw", [2, 128, NQ, 2 * HEAD_DIM])   # pair: [nope0|nope1|ropes]
    kvbw_ap = din("kvbw", [128, NL, HPC * (QK_NOPE + V_HEAD)])
    ow_ap = din("ow", [128, HPC, HIDDEN])
    mask_ap = din("mask", [128, 896])                 # 0/1 causal bank (bf16)
    ones128_ap = din("ones128", [128, 1])
    out_ap = nc.dram_tensor("out", [HIDDEN, T], bf, kind="ExternalOutput").ap()

    def eng(idx):
        return nc.scalar if idx % 2 else nc.vector

    def copy(e, out, in_):
        if e is nc.scalar:
            nc.scalar.copy(out, in_)
        else:
            nc.vector.tensor_copy(out, in_)

    with tile.TileContext(nc) as tc:
        with tc.tile_pool(name="consts", bufs=1) as consts, \
             tc.tile_pool(name="trans", bufs=3) as trans, \
             tc.tile_pool(name="dram", bufs=1, space="DRAM") as dram, \
             tc.tile_pool(name="act", bufs=1) as act:

            mask = consts.tile([128, 896], bf)
            nc.sync.dma_start(out=mask, in_=mask_ap)
            ones128 = consts.tile([128, 1], bf)
            nc.sync.dma_start(out=ones128, in_=ones128_ap)
            eps1 = consts.tile([1, 1], f32)
            nc.vector.memset(eps1, EPS)

            xq = act.tile([128, NQ, T], bf)
            xkv = act.tile([128, NKV, T], bf)
            rq_b = act.tile([128, T], f32)
            rkv_b = act.tile([128, T], f32)
            rkvT = act.tile([128, TT], f32)

            # ---- Stage A: xT via DMA-XBAR; xq = qaw.T@xT; xkv = kvaw.T@xT
            with tc.tile_pool(name="stageA", bufs=1) as pA, \
                 tc.tile_pool(name="wa", bufs=3) as pwa, \
                 tc.tile_pool(name="pB", bufs=1) as pB, \
                 tc.tile_pool(name="psumA", bufs=1, space="PSUM") as psumA:
                xT = pA.tile([128, KC, T], bf)
                for k in range(KC):
                    nc.sync.dma_start(
                        out=xT[:, k, :], in_=x_ap[:, ts(k, 128)], transpose=True
                    )

                for src_ap, ncols, dst in ((qaw_ap, NQ, xq), (kvaw_ap, NKV, xkv)):
                    for n in range(ncols):
                        wa = pwa.tile([128, KC, 128], bf, tag="wa", bufs=3)
                        nc.gpsimd.dma_start(out=wa, in_=src_ap[n])
                        for t in range(TQ):
                            psm = psumA.tile([128, 512], f32, tag="psm", bufs=3)
                            for kk in range(KC):
                                nc.tensor.matmul(
                                    out=psm,
                                    lhsT=wa[:, kk, :],
                                    rhs=xT[:, kk, ts(t, 512)],
                                    start=(kk == 0),
                                    stop=(kk == KC - 1),
                                )
                            copy(eng(n + t), dst[:, n, ts(t, 512)], psm)

                # ---- Stage B: rstd rows -> DRAM bounce -> partition broadcasts
                rstd_q = pB.tile([1, T], f32, tag="rstd_q", bufs=1)
                rstd_kv = pB.tile([1, T], f32, tag="rstd_kv", bufs=1)
                for t in range(TQ):
                    for src, nn, lora, rstd in (
                        (xq, NQ, Q_LORA, rstd_q),
                        (xkv, NL, KV_LORA, rstd_kv),
                    ):
                        psd = psumA.tile([1, 512], f32, tag="psd", bufs=2)
                        for n in range(nn):
                            sq = pB.tile([128, 512], bf, tag="sq", bufs=3)
                            nc.vector.tensor_mul(
                                sq, src[:, n, ts(t, 512)], src[:, n, ts(t, 512)]
                            )
                            nc.tensor.matmul(
                                out=psd, lhsT=ones128, rhs=sq,
                                start=(n == 0), stop=(n == nn - 1),
                            )
                        tmp = pB.tile([1, 512], f32, tag="tmp", bufs=1)
                        nc.scalar.activation(
                            out=tmp, in_=psd, func=AF.Sqrt, bias=eps1,
                            scale=1.0 / lora,
                        )
                        nc.vector.reciprocal_approx_fast(
                            out=rstd[:, ts(t, 512)], in_=tmp
                        )
                rstdq_d = dram.tile([1, T], f32)
                rkv_d = dram.tile([1, T], f32)
                nc.gpsimd.dma_start(out=rstdq_d, in_=rstd_q)
                nc.gpsimd.dma_start(out=rkv_d, in_=rstd_kv)
                nc.gpsimd.dma_start(out=rq_b, in_=rstdq_d.to_broadcast((128, T)))
                nc.gpsimd.dma_start(out=rkv_b, in_=rkv_d.to_broadcast((128, T)))
                nc.gpsimd.dma_start(
                    out=rkvT, in_=rkv_d.rearrange("o (tt p) -> (o p) tt", p=128)
                )

            kv_lat = xkv  # raw; norm applied at projection copy-out

            # ---- Stages D/E/F
            with tc.tile_pool(name="att", bufs=1) as patt, \
                 tc.tile_pool(name="owp", bufs=1) as powp, \
                 tc.tile_pool(name="psumD", bufs=1, space="PSUM") as psumD:
                qn = patt.tile([128, HPC, T], bf)
                qr = patt.tile([128, 2, T], bf)
                kn = patt.tile([128, HPC, T], bf)
                vv = patt.tile([128, TT, HPC * V_HEAD], bf)

                with tc.tile_pool(name="wD", bufs=1) as pw:
                    qbws = []
                    for pair in range(2):
                        qbw = pw.tile([128, NQ, 2 * HEAD_DIM], bf, tag="qbw",
                                      bufs=2, name=f"qbw{pair}")
                        nc.sync.dma_start(out=qbw, in_=qbw_ap[pair])
                        qbws.append(qbw)
                    kvbw = pw.tile([128, NL, HPC * (QK_NOPE + V_HEAD)], bf)
                    nc.sync.dma_start(out=kvbw, in_=kvbw_ap)
                    ow = powp.tile([128, HPC, HIDDEN], bf)
                    nc.sync.dma_start(out=ow, in_=ow_ap)

                    # ---- Stage D: q (x rstd_q), k_nope (x rstd_kv), v (x rstd_kv)
                    for pair in range(2):
                        qbw = qbws[pair]
                        for t in range(TQ):
                            for sub in range(3):  # nope0 | nope1 | rope pair
                                ps = psumD.tile([128, 512], f32, tag="psm", bufs=2)
                                for kk in range(NQ):
                                    nc.tensor.matmul(
                                        out=ps,
                                        lhsT=qbw[:, kk, ts(sub, 128)],
                                        rhs=xq[:, kk, ts(t, 512)],
                                        start=(kk == 0),
                                        stop=(kk == NQ - 1),
                                    )
                                dstv = (qn[:, 2 * pair, ts(t, 512)],
                                        qn[:, 2 * pair + 1, ts(t, 512)],
                                        qr[:, pair, ts(t, 512)])[sub]
                                nc.vector.tensor_mul(dstv, ps, rq_b[:, ts(t, 512)])
                    for h in range(HPC):
                        for t in range(TQ):
                            ps3 = psumD.tile([128, 512], f32, tag="psm", bufs=2)
                            for kk in range(NL):
                                nc.tensor.matmul(
                                    out=ps3,
                                    lhsT=kvbw[:, kk, ts(h, 256)][:, 0:128],
                                    rhs=kv_lat[:, kk, ts(t, 512)],
                                    start=(kk == 0),
                                    stop=(kk == NL - 1),
                                )
                            nc.vector.tensor_mul(
                                kn[:, h, ts(t, 512)], ps3, rkv_b[:, ts(t, 512)]
                            )
                    vcols = kvbw.rearrange(
                        "p kk (h two dv) -> p kk h two dv", h=HPC, two=2
                    )
                    for tt in range(TT):
                        psv = psumD.tile([128, 512], f32, tag="psm", bufs=2)
                        for kk in range(NL):
                            nc.tensor.matmul(
                                out=psv,
                                lhsT=kv_lat[:, kk, ts(tt, 128)],
                                rhs=vcols[:, kk, :, 1, :],
                                start=(kk == 0),
                                stop=(kk == NL - 1),
                            )
                        nc.scalar.mul(vv[:, tt, :], psv, mul=rkvT[:, tt:tt + 1])

                # ---- Stage E+F: causal attention; o_proj one chunk behind
                with tc.tile_pool(name="attn_i", bufs=2) as pai, \
                     tc.tile_pool(name="ob", bufs=2) as pob, \
                     tc.tile_pool(name="rdb", bufs=2) as prdb:
                    attn_tiles = []

                    def attention_chunk(i):
                        attn_i = pai.tile([128, HPC, 512], bf, tag="attn_i", bufs=2)
                        for h in range(HPC):
                            nj = 4 * i + 4
                            pso = psumD.tile([128, 512], f32, tag="pso", bufs=2)
                            psd = psumD.tile([1, 512], f32, tag="psd", bufs=1)
                            hp = 64 * (h % 2)

                            def consume(jc, ex, first, last):
                                nc.tensor.matmul(
                                    out=psd, lhsT=ones128, rhs=ex,
                                    start=first, stop=last,
                                )
                                nc.tensor.matmul(
                                    out=pso, lhsT=vv[:, jc, ts(h, V_HEAD)], rhs=ex,
                                    start=first, stop=last,
                                )

                            pending = []
                            for j in range(nj):
                                pss = psumD.tile([128, 512], f32, tag="pss", bufs=3)
                                nc.tensor.matmul(
                                    out=pss,
                                    lhsT=kn[:, h, ts(j, 128)],
                                    rhs=qn[:, h, ts(i, 512)],
                                    start=True,
                                    stop=False,
                                )
                                nc.tensor.matmul(
                                    out=pss,
                                    lhsT=xkv[hp:hp + 64, NL + h // 2, ts(j, 128)],
                                    rhs=qr[hp:hp + 64, h // 2, ts(i, 512)],
                                    start=False,
                                    stop=True,
                                )
                                while len(pending) > 1:
                                    jc, exc = pending.pop(0)
                                    consume(jc, exc, jc == 0, False)
                                ex = trans.tile([128, 512], bf, tag="ex", bufs=3)
                                nc.scalar.activation(out=ex, in_=pss, func=AF.Exp)
                                off = j * 128 - i * 512
                                if off >= 0:
                                    nc.vector.tensor_mul(
                                        ex, ex, mask[:, 384 - off:896 - off]
                                    )
                                pending.append((j, ex))
                            for jc, exc in pending:
                                consume(jc, exc, jc == 0, jc == nj - 1)

                            rd = trans.tile([1, 512], f32, tag="rd", bufs=1)
                            nc.vector.reciprocal_approx_fast(out=rd, in_=psd)
                            rd_bf = trans.tile([1, 512], bf, tag="rd_bf", bufs=2)
                            nc.scalar.copy(rd_bf, rd)
                            rd_d = dram.tile([1, 512], bf, tag="rd_d", bufs=3)
                            nc.gpsimd.dma_start(out=rd_d, in_=rd_bf)
                            rdb = prdb.tile([128, 512], bf, tag="rdb", bufs=2)
                            nc.sync.dma_start(
                                out=rdb, in_=rd_d.to_broadcast((128, 512))
                            )
                            nc.vector.tensor_mul(attn_i[:, h, :], pso, rdb)
                        attn_tiles.append(attn_i)

                    def oproj_chunk(i):
                        attn_i = attn_tiles[i]
                        for m in range(TT):
                            psf = psumD.tile([128, 512], f32, tag="psm", bufs=2)
                            for kk in range(HPC):
                                nc.tensor.matmul(
                                    out=psf,
                                    lhsT=ow[:, kk, ts(m, 128)],
                                    rhs=attn_i[:, kk, :],
                                    start=(kk == 0),
                                    stop=(kk == HPC - 1),
                                )
                            ob = pob.tile([128, 512], bf, tag="ob", bufs=2)
                            copy(eng(m), ob, psf)
                            nc.gpsimd.dma_start(
                                out=out_ap[ts(m, 128), ts(i, 512)], in_=ob
                            )

                    attention_chunk(0)
                    for i in range(1, TQ):
                        attention_chunk(i)
                        oproj_chunk(i - 1)
                    oproj_chunk(TQ - 1)

    nc.compile()
    return nc


def _tile_w(w):
    """[K, N] -> [N/128, 128, K/128, 128] so each col-block loads contiguously."""
    K, N = w.shape
    return np.ascontiguousarray(
        w.reshape(K // 128, 128, N // 128, 128).transpose(2, 1, 0, 3))


def _prep(inputs):
    x = np.asarray(inputs["hidden_states"], np.float32)
    qaw = np.asarray(inputs["q_a_w"], np.float32)
    qalw = np.asarray(inputs["q_a_ln_w"], np.float32)
    qbw = np.asarray(inputs["q_b_w"], np.float32)
    kvaw = np.asarray(inputs["kv_a_w"], np.float32)
    kvlw = np.asarray(inputs["kv_a_ln_w"], np.float32)
    kvbw = np.asarray(inputs["kv_b_w"], np.float32)
    ow = np.asarray(inputs["o_w"], np.float32)

    scale = 1.0 / np.sqrt(np.float32(HEAD_DIM))
    qbw_f = (qbw * qalw[:, None] * scale).astype(BF16)
    kvbw_f = (kvbw * kvlw[:, None]).astype(BF16)
    qaw_t = _tile_w(qaw.astype(BF16))               # [NQ, 128, KC, 128]

    r = np.arange(128)[:, None]
    j = np.arange(896)[None, :]
    mask = np.where((j - 384) >= r, 1.0, 0.0).astype(BF16)
    ones128 = np.ones((128, 1), BF16)

    def lat_tiled(w):  # [KV_LORA, N] -> [128, NL, N] (p, kk, n)
        return np.ascontiguousarray(w.reshape(NL, 128, -1).transpose(1, 0, 2))

    in_maps = []
    for c in range(NCORES):
        b, g = c // 4, c % 4
        qbw_g = qbw_f[:, g * HPC * HEAD_DIM:(g + 1) * HPC * HEAD_DIM]
        pairs = []
        for pair in range(HPC // 2):
            h0, h1 = 2 * pair, 2 * pair + 1
            cols = np.concatenate([
                qbw_g[:, h0 * HEAD_DIM:h0 * HEAD_DIM + QK_NOPE],
                qbw_g[:, h1 * HEAD_DIM:h1 * HEAD_DIM + QK_NOPE],
                qbw_g[:, h0 * HEAD_DIM + QK_NOPE:(h0 + 1) * HEAD_DIM],
                qbw_g[:, h1 * HEAD_DIM + QK_NOPE:(h1 + 1) * HEAD_DIM],
            ], axis=1)  # [Q_LORA, 384]
            pairs.append(cols.reshape(NQ, 128, 384).transpose(1, 0, 2))
        qbw_c = np.ascontiguousarray(np.stack(pairs))   # [2, 128, NQ, 384]

        kvaw_g = np.concatenate(
            [kvaw[:, :KV_LORA],
             kvaw[:, KV_LORA + g * HPC * QK_ROPE:
                  KV_LORA + (g + 1) * HPC * QK_ROPE]], axis=1).astype(BF16)

        in_maps.append({
            "x": x[b].astype(BF16),
            "qaw": qaw_t,
            "kvaw": _tile_w(kvaw_g),
            "qbw": qbw_c,
            "kvbw": lat_tiled(kvbw_f[:, g * HPC * 256:(g + 1) * HPC * 256]),
            "ow": np.ascontiguousarray(
                ow[g * HPC * V_HEAD:(g + 1) * HPC * V_HEAD]
                .astype(BF16).reshape(HPC, 128, HIDDEN).transpose(1, 0, 2)),
            "mask": mask,
            "ones128": ones128,
        })
    return in_maps


def kernel(**inputs):
    from concourse.bass_utils import run_bass_kernel_spmd

    if "nc" not in _CACHE:
        _CACHE["nc"] = _build()
    nc = _CACHE["nc"]
    in_maps = _prep(inputs)
    res = run_bass_kernel_spmd(nc, in_maps, core_ids=list(range(NCORES)),
                               **_CACHE.get("run_kwargs", {}))
    _CACHE["last_results"] = res
    out = np.zeros((B, T, HIDDEN), np.float32)
    for c in range(NCORES):
        out[c // 4] += np.asarray(res.results[c]["out"], np.float32).T
    return out


# revision 39
# speedup vs baseline: 1.1429x; 1.0411x over previous
"""MLA (multi-head latent attention) forward on 8 TRN2 NeuronCores.

Sharding: 2-way data-parallel over batch x 4-way tensor-parallel over heads.
Core c handles batch b=c//4 and heads 4g..4g+3 where g=c%4. Each core runs an
identical SPMD program on its shard; the host sums the 4 partial outputs per
batch (the o_proj contribution of each head group) and transposes.

Layout: activations are feature-major ([feature, token]) so every matmul
contracts over the partition dim; x is transposed once by the DMA XBAR.
Probabilities are computed transposed (s[tk, tq]) so softmax needs no
max-subtraction (scores are bounded ~6) and P@V contracts naturally;
denominators come from ones-matmuls + fast reciprocal + DRAM-bounce
partition-broadcast. RMSNorm scaling is per-token so it commutes with the
B-projections: both q and kv normalizations are applied at copy-out of the
projected tensors, keeping the whole norm pipeline off the TensorE stream.
Weights are pre-tiled on the host so every weight DMA is contiguous.
"""

import numpy as np
import ml_dtypes

B, T, HIDDEN = 2, 2048, 2048
NUM_HEADS = 16
QK_NOPE, QK_ROPE, HEAD_DIM, V_HEAD = 128, 64, 192, 128
KV_LORA, Q_LORA = 512, 1536
EPS = 1e-6
NCORES = 8
HPC = 4  # heads per core

KC = HIDDEN // 128
TT = T // 128
TQ = T // 512
NQ = Q_LORA // 128
NKV = (KV_LORA + HPC * QK_ROPE) // 128
NL = KV_LORA // 128

BF16 = ml_dtypes.bfloat16

_CACHE = {}


def _build():
    import concourse.bass as bass
    import concourse.tile as tile
    from concourse import bacc, mybir
    from concourse.bass import ts

    f32 = mybir.dt.float32
    bf = mybir.dt.bfloat16
    AF = mybir.ActivationFunctionType

    nc = bacc.Bacc(
        "TRN2",
        target_bir_lowering=False,
        debug=False,
        enable_asserts=True,
        num_devices=NCORES,
    )

    def din(name, shape, dt=bf):
        return nc.dram_tensor(name, shape, dt, kind="ExternalInput").ap()

    # weights pre-tiled on host: contiguous per-tile DMA loads
    x_ap = din("x", [T, HIDDEN])                      # [t, d] (XBAR-transposed)
    qaw_ap = din("qaw", [NQ, 128, KC, 128])           # per col-block [p, kk, c]
    kvaw_ap = din("kvaw", [NKV, 128, KC, 128])
    qbw_ap = din("qbw", [2, 128, NQ, 2 * HEAD_DIM])   # pair: [nope0|nope1|ropes]
    kvbw_ap = din("kvbw", [128, NL, HPC * (QK_NOPE + V_HEAD)])
    ow_ap = din("ow", [128, HPC, HIDDEN])
    mask_ap = din("mask", [128, 896])                 # 0/1 causal bank (bf16)
    ones128_ap = din("ones128", [128, 1])
    out_ap = nc.dram_tensor("out", [HIDDEN, T], bf, kind="ExternalOutput").ap()

    def eng(idx):
        return nc.scalar if idx % 2 else nc.vector

    def copy(e, out, in_):
        if e is nc.scalar:
            nc.scalar.copy(out, in_)
        else:
            nc.vector.tensor_copy(out, in_)

    with tile.TileContext(nc) as tc:
        with tc.tile_pool(name="consts", bufs=1) as consts, \
             tc.tile_pool(name="trans", bufs=3) as trans, \
             tc.tile_pool(name="dram", bufs=1, space="DRAM") as dram, \
             tc.tile_pool(name="act", bufs=1) as act:

            mask = consts.tile([128, 896], bf)
            nc.sync.dma_start(out=mask, in_=mask_ap)
            ones128 = consts.tile([128, 1], bf)
            nc.sync.dma_start(out=ones128, in_=ones128_ap)
            eps1 = consts.tile([1, 1], f32)
            nc.vector.memset(eps1, EPS)

            xq = act.tile([128, NQ, T], bf)
            xkv = act.tile([128, NKV, T], bf)
            rq_b = act.tile([128, T], f32)
            rkv_b = act.tile([128, T], f32)
            rkvT = act.tile([128, TT], f32)

            # ---- Stage A: xT via DMA-XBAR; xq = qaw.T@xT; xkv = kvaw.T@xT
            with tc.tile_pool(name="stageA", bufs=1) as pA, \
                 tc.tile_pool(name="wa", bufs=3) as pwa, \
                 tc.tile_pool(name="pB", bufs=1) as pB, \
                 tc.tile_pool(name="psumA", bufs=1, space="PSUM") as psumA:
                xT = pA.tile([128, KC, T], bf)
                for k in range(KC):
                    nc.sync.dma_start(
                        out=xT[:, k, :], in_=x_ap[:, ts(k, 128)], transpose=True
                    )

                for src_ap, ncols, dst in ((qaw_ap, NQ, xq), (kvaw_ap, NKV, xkv)):
                    for n in range(ncols):
                        wa = pwa.tile([128, KC, 128], bf, tag="wa", bufs=3)
                        nc.gpsimd.dma_start(out=wa, in_=src_ap[n])
                        for t in range(TQ):
                            psm = psumA.tile([128, 512], f32, tag="psm", bufs=3)
                            for kk in range(KC):
                                nc.tensor.matmul(
                                    out=psm,
                                    lhsT=wa[:, kk, :],
                                    rhs=xT[:, kk, ts(t, 512)],
                                    start=(kk == 0),
                                    stop=(kk == KC - 1),
                                )
                            nc.scalar.copy(dst[:, n, ts(t, 512)], psm)

                # ---- Stage B: rstd rows -> DRAM bounce -> partition broadcasts
                rstd_q = pB.tile([1, T], f32, tag="rstd_q", bufs=1)
                rstd_kv = pB.tile([1, T], f32, tag="rstd_kv", bufs=1)
                for t in range(TQ):
                    for src, nn, lora, rstd in (
                        (xq, NQ, Q_LORA, rstd_q),
                        (xkv, NL, KV_LORA, rstd_kv),
                    ):
                        psd = psumA.tile([1, 512], f32, tag="psd", bufs=2)
                        for n in range(nn):
                            sq = pB.tile([128, 512], bf, tag="sq", bufs=14)
                            nc.vector.tensor_mul(
                                sq, src[:, n, ts(t, 512)], src[:, n, ts(t, 512)]
                            )
                            nc.tensor.matmul(
                                out=psd, lhsT=ones128, rhs=sq,
                                start=(n == 0), stop=(n == nn - 1),
                            )
                        tmp = pB.tile([1, 512], f32, tag="tmp", bufs=1)
                        nc.scalar.activation(
                            out=tmp, in_=psd, func=AF.Sqrt, bias=eps1,
                            scale=1.0 / lora,
                        )
                        nc.vector.reciprocal_approx_fast(
                            out=rstd[:, ts(t, 512)], in_=tmp
                        )
                rstdq_d = dram.tile([1, T], f32)
                rkv_d = dram.tile([1, T], f32)
                nc.gpsimd.dma_start(out=rstdq_d, in_=rstd_q)
                nc.gpsimd.dma_start(out=rkv_d, in_=rstd_kv)
                nc.gpsimd.dma_start(out=rq_b, in_=rstdq_d.to_broadcast((128, T)))
                nc.gpsimd.dma_start(out=rkv_b, in_=rkv_d.to_broadcast((128, T)))
                nc.gpsimd.dma_start(
                    out=rkvT, in_=rkv_d.rearrange("o (tt p) -> (o p) tt", p=128)
                )

            kv_lat = xkv  # raw; norm applied at projection copy-out

            # ---- Stages D/E/F
            with tc.tile_pool(name="att", bufs=1) as patt, \
                 tc.tile_pool(name="owp", bufs=1) as powp, \
                 tc.tile_pool(name="psumD", bufs=1, space="PSUM") as psumD:
                qn = patt.tile([128, HPC, T], bf)
                qr = patt.tile([128, 2, T], bf)
                kn = patt.tile([128, HPC, T], bf)
                vv = patt.tile([128, TT, HPC * V_HEAD], bf)

                with tc.tile_pool(name="wD", bufs=1) as pw:
                    qbws = []
                    for pair in range(2):
                        qbw = pw.tile([128, NQ, 2 * HEAD_DIM], bf, tag="qbw",
                                      bufs=2, name=f"qbw{pair}")
                        nc.sync.dma_start(out=qbw, in_=qbw_ap[pair])
                        qbws.append(qbw)
                    kvbw = pw.tile([128, NL, HPC * (QK_NOPE + V_HEAD)], bf)
                    nc.sync.dma_start(out=kvbw, in_=kvbw_ap)
                    ow = powp.tile([128, HPC, HIDDEN], bf)
                    nc.sync.dma_start(out=ow, in_=ow_ap)

                    # ---- Stage D: q (x rstd_q), k_nope (x rstd_kv), v (x rstd_kv)
                    for pair in range(2):
                        qbw = qbws[pair]
                        for t in range(TQ):
                            for sub in range(3):  # nope0 | nope1 | rope pair
                                ps = psumD.tile([128, 512], f32, tag="psm", bufs=2)
                                for kk in range(NQ):
                                    nc.tensor.matmul(
                                        out=ps,
                                        lhsT=qbw[:, kk, ts(sub, 128)],
                                        rhs=xq[:, kk, ts(t, 512)],
                                        start=(kk == 0),
                                        stop=(kk == NQ - 1),
                                    )
                                dstv = (qn[:, 2 * pair, ts(t, 512)],
                                        qn[:, 2 * pair + 1, ts(t, 512)],
                                        qr[:, pair, ts(t, 512)])[sub]
                                nc.vector.tensor_mul(dstv, ps, rq_b[:, ts(t, 512)])
                    for h in range(HPC):
                        for t in range(TQ):
                            ps3 = psumD.tile([128, 512], f32, tag="psm", bufs=2)
                            for kk in range(NL):
                                nc.tensor.matmul(
                                    out=ps3,
                                    lhsT=kvbw[:, kk, ts(h, 256)][:, 0:128],
                                    rhs=kv_lat[:, kk, ts(t, 512)],
                                    start=(kk == 0),
                                    stop=(kk == NL - 1),
                                )
                            nc.vector.tensor_mul(
                                kn[:, h, ts(t, 512)], ps3, rkv_b[:, ts(t, 512)]
                            )
                    vcols = kvbw.rearrange(
                        "p kk (h two dv) -> p kk h two dv", h=HPC, two=2
                    )
                    for tt in range(TT):
                        psv = psumD.tile([128, 512], f32, tag="psm", bufs=2)
                        for kk in range(NL):
                            nc.tensor.matmul(
                                out=psv,
                                lhsT=kv_lat[:, kk, ts(tt, 128)],
                                rhs=vcols[:, kk, :, 1, :],
                                start=(kk == 0),
                                stop=(kk == NL - 1),
                            )
                        nc.scalar.mul(vv[:, tt, :], psv, mul=rkvT[:, tt:tt + 1])

                # ---- Stage E+F: causal attention; o_proj one chunk behind
                with tc.tile_pool(name="attn_i", bufs=2) as pai, \
                     tc.tile_pool(name="ob", bufs=2) as pob, \
                     tc.tile_pool(name="rdb", bufs=2) as prdb:
                    attn_tiles = []

                    def attention_chunk(i):
                        attn_i = pai.tile([128, HPC, 512], bf, tag="attn_i", bufs=2)
                        for h in range(HPC):
                            nj = 4 * i + 4
                            pso = psumD.tile([128, 512], f32, tag="pso", bufs=2)
                            psd = psumD.tile([1, 512], f32, tag="psd", bufs=1)
                            hp = 64 * (h % 2)

                            def consume(jc, ex, first, last):
                                nc.tensor.matmul(
                                    out=psd, lhsT=ones128, rhs=ex,
                                    start=first, stop=last,
                                )
                                nc.tensor.matmul(
                                    out=pso, lhsT=vv[:, jc, ts(h, V_HEAD)], rhs=ex,
                                    start=first, stop=last,
                                )

                            pending = []
                            for j in range(nj):
                                pss = psumD.tile([128, 512], f32, tag="pss", bufs=3)
                                nc.tensor.matmul(
                                    out=pss,
                                    lhsT=kn[:, h, ts(j, 128)],
                                    rhs=qn[:, h, ts(i, 512)],
                                    start=True,
                                    stop=False,
                                )
                                nc.tensor.matmul(
                                    out=pss,
                                    lhsT=xkv[hp:hp + 64, NL + h // 2, ts(j, 128)],
                                    rhs=qr[hp:hp + 64, h // 2, ts(i, 512)],
                                    start=False,
                                    stop=True,
                                )
                                while len(pending) > 1:
                                    jc, exc = pending.pop(0)
                                    consume(jc, exc, jc == 0, False)
                                ex = trans.tile([128, 512], bf, tag="ex", bufs=3)
                                nc.scalar.activation(out=ex, in_=pss, func=AF.Exp)
                                off = j * 128 - i * 512
                                if off >= 0:
                                    nc.vector.tensor_mul(
                                        ex, ex, mask[:, 384 - off:896 - off]
                                    )
                                pending.append((j, ex))
                            for jc, exc in pending:
                                consume(jc, exc, jc == 0, jc == nj - 1)

                            rd = trans.tile([1, 512], f32, tag="rd", bufs=1)
                            nc.vector.reciprocal_approx_fast(out=rd, in_=psd)
                            rd_bf = trans.tile([1, 512], bf, tag="rd_bf", bufs=2)
                            nc.scalar.copy(rd_bf, rd)
                            rd_d = dram.tile([1, 512], bf, tag="rd_d", bufs=3)
                            nc.gpsimd.dma_start(out=rd_d, in_=rd_bf)
                            rdb = prdb.tile([128, 512], bf, tag="rdb", bufs=2)
                            nc.sync.dma_start(
                                out=rdb, in_=rd_d.to_broadcast((128, 512))
                            )
                            nc.vector.tensor_mul(attn_i[:, h, :], pso, rdb)
                        attn_tiles.append(attn_i)

                    def oproj_chunk(i):
                        attn_i = attn_tiles[i]
                        for m in range(TT):
                            psf = psumD.tile([128, 512], f32, tag="psm", bufs=2)
                            for kk in range(HPC):
                                nc.tensor.matmul(
                                    out=psf,
                                    lhsT=ow[:, kk, ts(m, 128)],
                                    rhs=attn_i[:, kk, :],
                                    start=(kk == 0),
                                    stop=(kk == HPC - 1),
                                )
                            ob = pob.tile([128, 512], bf, tag="ob", bufs=3)
                            copy(eng(m), ob, psf)
                            nc.sync.dma_start(
                                out=out_ap[ts(m, 128), ts(i, 512)], in_=ob
                            )

                    attention_chunk(0)
                    for i in range(1, TQ):
                        attention_chunk(i)
                        oproj_chunk(i - 1)
                    oproj_chunk(TQ - 1)

    nc.compile()
    return nc


def _tile_w(w):
    """[K, N] -> [N/128, 128, K/128, 128] so each col-block loads contiguously."""
    K, N = w.shape
    return np.ascontiguousarray(
        w.reshape(K // 128, 128, N // 128, 128).transpose(2, 1, 0, 3))


def _prep(inputs):
    x = np.asarray(inputs["hidden_states"], np.float32)
    qaw = np.asarray(inputs["q_a_w"], np.float32)
    qalw = np.asarray(inputs["q_a_ln_w"], np.float32)
    qbw = np.asarray(inputs["q_b_w"], np.float32)
    kvaw = np.asarray(inputs["kv_a_w"], np.float32)
    kvlw = np.asarray(inputs["kv_a_ln_w"], np.float32)
    kvbw = np.asarray(inputs["kv_b_w"], np.float32)
    ow = np.asarray(inputs["o_w"], np.float32)

    scale = 1.0 / np.sqrt(np.float32(HEAD_DIM))
    qbw_f = (qbw * qalw[:, None] * scale).astype(BF16)
    kvbw_f = (kvbw * kvlw[:, None]).astype(BF16)
    qaw_t = _tile_w(qaw.astype(BF16))               # [NQ, 128, KC, 128]

    r = np.arange(128)[:, None]
    j = np.arange(896)[None, :]
    mask = np.where((j - 384) >= r, 1.0, 0.0).astype(BF16)
    ones128 = np.ones((128, 1), BF16)

    def lat_tiled(w):  # [KV_LORA, N] -> [128, NL, N] (p, kk, n)
        return np.ascontiguousarray(w.reshape(NL, 128, -1).transpose(1, 0, 2))

    in_maps = []
    for c in range(NCORES):
        b, g = c // 4, c % 4
        qbw_g = qbw_f[:, g * HPC * HEAD_DIM:(g + 1) * HPC * HEAD_DIM]
        pairs = []
        for pair in range(HPC // 2):
            h0, h1 = 2 * pair, 2 * pair + 1
            cols = np.concatenate([
                qbw_g[:, h0 * HEAD_DIM:h0 * HEAD_DIM + QK_NOPE],
                qbw_g[:, h1 * HEAD_DIM:h1 * HEAD_DIM + QK_NOPE],
                qbw_g[:, h0 * HEAD_DIM + QK_NOPE:(h0 + 1) * HEAD_DIM],
                qbw_g[:, h1 * HEAD_DIM + QK_NOPE:(h1 + 1) * HEAD_DIM],
            ], axis=1)  # [Q_LORA, 384]
            pairs.append(cols.reshape(NQ, 128, 384).transpose(1, 0, 2))
        qbw_c = np.ascontiguousarray(np.stack(pairs))   # [2, 128, NQ, 384]

        kvaw_g = np.concatenate(
            [kvaw[:, :KV_LORA],
             kvaw[:, KV_LORA + g * HPC * QK_ROPE:
                  KV_LORA + (g + 1) * HPC * QK_ROPE]], axis=1).astype(BF16)

        in_maps.append({
            "x": x[b].astype(BF16),
            "qaw": qaw_t,
            "kvaw": _tile_w(kvaw_g),
            "qbw": qbw_c,
            "kvbw": lat_tiled(kvbw_f[:, g * HPC * 256:(g + 1) * HPC * 256]),
            "ow": np.ascontiguousarray(
                ow[g * HPC * V_HEAD:(g + 1) * HPC * V_HEAD]
                .astype(BF16).reshape(HPC, 128, HIDDEN).transpose(1, 0, 2)),
            "mask": mask,
            "ones128": ones128,
        })
    return in_maps


def kernel(**inputs):
    from concourse.bass_utils import run_bass_kernel_spmd

    if "nc" not in _CACHE:
        _CACHE["nc"] = _build()
    nc = _CACHE["nc"]
    in_maps = _prep(inputs)
    res = run_bass_kernel_spmd(nc, in_maps, core_ids=list(range(NCORES)),
                               **_CACHE.get("run_kwargs", {}))
    _CACHE["last_results"] = res
    out = np.zeros((B, T, HIDDEN), np.float32)
    for c in range(NCORES):
        out[c // 4] += np.asarray(res.results[c]["out"], np.float32).T
    return out


# revision 41
# speedup vs baseline: 1.1436x; 1.0006x over previous
"""MLA (multi-head latent attention) forward on 8 TRN2 NeuronCores.

Sharding: 2-way data-parallel over batch x 4-way tensor-parallel over heads.
Core c handles batch b=c//4 and heads 4g..4g+3 where g=c%4. Each core runs an
identical SPMD program on its shard; the host sums the 4 partial outputs per
batch (the o_proj contribution of each head group) and transposes.

Layout: activations are feature-major ([feature, token]) so every matmul
contracts over the partition dim; x is transposed once by the DMA XBAR.
Probabilities are computed transposed (s[tk, tq]) so softmax needs no
max-subtraction (scores are bounded ~6) and P@V contracts naturally;
denominators come from ones-matmuls + fast reciprocal + DRAM-bounce
partition-broadcast. RMSNorm scaling is per-token so it commutes with the
B-projections: both q and kv normalizations are applied at copy-out of the
projected tensors, keeping the whole norm pipeline off the TensorE stream.
Weights are pre-tiled on the host so every weight DMA is contiguous.
"""

import numpy as np
import ml_dtypes

B, T, HIDDEN = 2, 2048, 2048
NUM_HEADS = 16
QK_NOPE, QK_ROPE, HEAD_DIM, V_HEAD = 128, 64, 192, 128
KV_LORA, Q_LORA = 512, 1536
EPS = 1e-6
NCORES = 8
HPC = 4  # heads per core

KC = HIDDEN // 128
TT = T // 128
TQ = T // 512
NQ = Q_LORA // 128
NKV = (KV_LORA + HPC * QK_ROPE) // 128
NL = KV_LORA // 128

BF16 = ml_dtypes.bfloat16

_CACHE = {}


def _build():
    import concourse.bass as bass
    import concourse.tile as tile
    from concourse import bacc, mybir
    from concourse.bass import ts

    f32 = mybir.dt.float32
    bf = mybir.dt.bfloat16
    AF = mybir.ActivationFunctionType

    nc = bacc.Bacc(
        "TRN2",
        target_bir_lowering=False,
        debug=False,
        enable_asserts=True,
        num_devices=NCORES,
    )

    def din(name, shape, dt=bf):
        return nc.dram_tensor(name, shape, dt, kind="ExternalInput").ap()

    # weights pre-tiled on host: contiguous per-tile DMA loads
    x_ap = din("x", [T, HIDDEN])                      # [t, d] (XBAR-transposed)
    qaw_ap = din("qaw", [NQ, 128, KC, 128])           # per col-block [p, kk, c]
    kvaw_ap = din("kvaw", [NKV, 128, KC, 128])
    qbw_ap = din("qbw", [2, 128, NQ, 2 * HEAD_DIM])   # pair: [nope0|nope1|ropes]
    kvbw_ap = din("kvbw", [128, NL, HPC * (QK_NOPE + V_HEAD)])
    ow_ap = din("ow", [128, HPC, HIDDEN])
    mask_ap = din("mask", [128, 896])                 # 0/1 causal bank (bf16)
    ones128_ap = din("ones128", [128, 1])
    out_ap = nc.dram_tensor("out", [HIDDEN, T], bf, kind="ExternalOutput").ap()

    def eng(idx):
        return nc.scalar if idx % 2 else nc.vector

    def copy(e, out, in_):
        if e is nc.scalar:
            nc.scalar.copy(out, in_)
        else:
            nc.vector.tensor_copy(out, in_)

    with tile.TileContext(nc) as tc:
        with tc.tile_pool(name="consts", bufs=1) as consts, \
             tc.tile_pool(name="trans", bufs=3) as trans, \
             tc.tile_pool(name="dram", bufs=1, space="DRAM") as dram, \
             tc.tile_pool(name="act", bufs=1) as act:

            mask = consts.tile([128, 896], bf)
            nc.sync.dma_start(out=mask, in_=mask_ap)
            ones128 = consts.tile([128, 1], bf)
            nc.sync.dma_start(out=ones128, in_=ones128_ap)
            eps1 = consts.tile([1, 1], f32)
            nc.vector.memset(eps1, EPS)

            xq = act.tile([128, NQ, T], bf)
            xkv = act.tile([128, NKV, T], bf)
            rq_b = act.tile([128, T], f32)
            rkv_b = act.tile([128, T], f32)
            rkvT = act.tile([128, TT], f32)

            # ---- Stage A: xT via DMA-XBAR; xq = qaw.T@xT; xkv = kvaw.T@xT
            with tc.tile_pool(name="stageA", bufs=1) as pA, \
                 tc.tile_pool(name="wa", bufs=3) as pwa, \
                 tc.tile_pool(name="pB", bufs=1) as pB, \
                 tc.tile_pool(name="psumA", bufs=1, space="PSUM") as psumA:
                xT = pA.tile([128, KC, T], bf)
                for k in range(KC):
                    nc.sync.dma_start(
                        out=xT[:, k, :], in_=x_ap[:, ts(k, 128)], transpose=True
                    )

                for src_ap, ncols, dst in ((qaw_ap, NQ, xq), (kvaw_ap, NKV, xkv)):
                    for n in range(ncols):
                        wa = pwa.tile([128, KC, 128], bf, tag="wa", bufs=3)
                        nc.gpsimd.dma_start(out=wa, in_=src_ap[n])
                        for t in range(TQ):
                            psm = psumA.tile([128, 512], f32, tag="psm", bufs=3)
                            for kk in range(KC):
                                nc.tensor.matmul(
                                    out=psm,
                                    lhsT=wa[:, kk, :],
                                    rhs=xT[:, kk, ts(t, 512)],
                                    start=(kk == 0),
                                    stop=(kk == KC - 1),
                                )
                            nc.scalar.copy(dst[:, n, ts(t, 512)], psm)

                # ---- Stage B: rstd rows -> DRAM bounce -> partition broadcasts
                rstd_q = pB.tile([1, T], f32, tag="rstd_q", bufs=1)
                rstd_kv = pB.tile([1, T], f32, tag="rstd_kv", bufs=1)
                for t in range(TQ):
                    for src, nn, lora, rstd in (
                        (xq, NQ, Q_LORA, rstd_q),
                        (xkv, NL, KV_LORA, rstd_kv),
                    ):
                        psd = psumA.tile([1, 512], f32, tag="psd", bufs=2)
                        for n in range(nn):
                            sq = pB.tile([128, 512], bf, tag="sq", bufs=13)
                            nc.vector.tensor_mul(
                                sq, src[:, n, ts(t, 512)], src[:, n, ts(t, 512)]
                            )
                            nc.tensor.matmul(
                                out=psd, lhsT=ones128, rhs=sq,
                                start=(n == 0), stop=(n == nn - 1),
                            )
                        tmp = pB.tile([1, 512], f32, tag="tmp", bufs=1)
                        nc.scalar.activation(
                            out=tmp, in_=psd, func=AF.Sqrt, bias=eps1,
                            scale=1.0 / lora,
                        )
                        nc.vector.reciprocal_approx_fast(
                            out=rstd[:, ts(t, 512)], in_=tmp
                        )
                rstdq_d = dram.tile([1, T], f32)
                rkv_d = dram.tile([1, T], f32)
                nc.gpsimd.dma_start(out=rstdq_d, in_=rstd_q)
                nc.gpsimd.dma_start(out=rkv_d, in_=rstd_kv)
                nc.gpsimd.dma_start(out=rq_b, in_=rstdq_d.to_broadcast((128, T)))
                nc.gpsimd.dma_start(out=rkv_b, in_=rkv_d.to_broadcast((128, T)))
                nc.gpsimd.dma_start(
                    out=rkvT, in_=rkv_d.rearrange("o (tt p) -> (o p) tt", p=128)
                )

            kv_lat = xkv  # raw; norm applied at projection copy-out

            # ---- Stages D/E/F
            with tc.tile_pool(name="att", bufs=1) as patt, \
                 tc.tile_pool(name="owp", bufs=1) as powp, \
                 tc.tile_pool(name="psumD", bufs=1, space="PSUM") as psumD:
                qn = patt.tile([128, HPC, T], bf)
                qr = patt.tile([128, 2, T], bf)
                kn = patt.tile([128, HPC, T], bf)
                vv = patt.tile([128, TT, HPC * V_HEAD], bf)

                with tc.tile_pool(name="wD", bufs=1) as pw:
                    qbws = []
                    for pair in range(2):
                        qbw = pw.tile([128, NQ, 2 * HEAD_DIM], bf, tag="qbw",
                                      bufs=2, name=f"qbw{pair}")
                        nc.sync.dma_start(out=qbw, in_=qbw_ap[pair])
                        qbws.append(qbw)
                    kvbw = pw.tile([128, NL, HPC * (QK_NOPE + V_HEAD)], bf)
                    nc.sync.dma_start(out=kvbw, in_=kvbw_ap)
                    ow = powp.tile([128, HPC, HIDDEN], bf)
                    nc.sync.dma_start(out=ow, in_=ow_ap)

                    # ---- Stage D: q (x rstd_q), k_nope (x rstd_kv), v (x rstd_kv)
                    for pair in range(2):
                        qbw = qbws[pair]
                        for t in range(TQ):
                            for sub in range(3):  # nope0 | nope1 | rope pair
                                ps = psumD.tile([128, 512], f32, tag="psm", bufs=2)
                                for kk in range(NQ):
                                    nc.tensor.matmul(
                                        out=ps,
                                        lhsT=qbw[:, kk, ts(sub, 128)],
                                        rhs=xq[:, kk, ts(t, 512)],
                                        start=(kk == 0),
                                        stop=(kk == NQ - 1),
                                    )
                                dstv = (qn[:, 2 * pair, ts(t, 512)],
                                        qn[:, 2 * pair + 1, ts(t, 512)],
                                        qr[:, pair, ts(t, 512)])[sub]
                                nc.vector.tensor_mul(dstv, ps, rq_b[:, ts(t, 512)])
                    for h in range(HPC):
                        for t in range(TQ):
                            ps3 = psumD.tile([128, 512], f32, tag="psm", bufs=2)
                            for kk in range(NL):
                                nc.tensor.matmul(
                                    out=ps3,
                                    lhsT=kvbw[:, kk, ts(h, 256)][:, 0:128],
                                    rhs=kv_lat[:, kk, ts(t, 512)],
                                    start=(kk == 0),
                                    stop=(kk == NL - 1),
                                )
                            nc.vector.tensor_mul(
                                kn[:, h, ts(t, 512)], ps3, rkv_b[:, ts(t, 512)]
                            )
                    vcols = kvbw.rearrange(
                        "p kk (h two dv) -> p kk h two dv", h=HPC, two=2
                    )
                    for tt in range(TT):
                        psv = psumD.tile([128, 512], f32, tag="psm", bufs=2)
                        for kk in range(NL):
                            nc.tensor.matmul(
                                out=psv,
                                lhsT=kv_lat[:, kk, ts(tt, 128)],
                                rhs=vcols[:, kk, :, 1, :],
                                start=(kk == 0),
                                stop=(kk == NL - 1),
                            )
                        nc.scalar.mul(vv[:, tt, :], psv, mul=rkvT[:, tt:tt + 1])

                # ---- Stage E+F: causal attention; o_proj one chunk behind
                with tc.tile_pool(name="attn_i", bufs=2) as pai, \
                     tc.tile_pool(name="ob", bufs=2) as pob, \
                     tc.tile_pool(name="rdb", bufs=2) as prdb:
                    attn_tiles = []

                    def attention_chunk(i):
                        attn_i = pai.tile([128, HPC, 512], bf, tag="attn_i", bufs=2)
                        for h in range(HPC):
                            nj = 4 * i + 4
                            pso = psumD.tile([128, 512], f32, tag="pso", bufs=2)
                            psd = psumD.tile([1, 512], f32, tag="psd", bufs=1)
                            hp = 64 * (h % 2)

                            def consume_batch(batch, last):
                                for jc, exc in batch:
                                    nc.tensor.matmul(
                                        out=psd, lhsT=ones128, rhs=exc,
                                        start=(jc == 0),
                                        stop=(last and jc == batch[-1][0]),
                                    )
                                for jc, exc in batch:
                                    nc.tensor.matmul(
                                        out=pso, lhsT=vv[:, jc, ts(h, V_HEAD)],
                                        rhs=exc,
                                        start=(jc == 0),
                                        stop=(last and jc == batch[-1][0]),
                                    )

                            pending = []
                            for j in range(nj):
                                pss = psumD.tile([128, 512], f32, tag="pss", bufs=3)
                                nc.tensor.matmul(
                                    out=pss,
                                    lhsT=kn[:, h, ts(j, 128)],
                                    rhs=qn[:, h, ts(i, 512)],
                                    start=True,
                                    stop=False,
                                )
                                nc.tensor.matmul(
                                    out=pss,
                                    lhsT=xkv[hp:hp + 64, NL + h // 2, ts(j, 128)],
                                    rhs=qr[hp:hp + 64, h // 2, ts(i, 512)],
                                    start=False,
                                    stop=True,
                                )
                                if len(pending) == 4:
                                    consume_batch(pending, False)
                                    pending = []
                                ex = trans.tile([128, 512], bf, tag="ex", bufs=6)
                                nc.scalar.activation(out=ex, in_=pss, func=AF.Exp)
                                off = j * 128 - i * 512
                                if off >= 0:
                                    nc.vector.tensor_mul(
                                        ex, ex, mask[:, 384 - off:896 - off]
                                    )
                                pending.append((j, ex))
                            if pending:
                                consume_batch(pending, True)

                            rd = trans.tile([1, 512], f32, tag="rd", bufs=1)
                            nc.vector.reciprocal_approx_fast(out=rd, in_=psd)
                            rd_bf = trans.tile([1, 512], bf, tag="rd_bf", bufs=2)
                            nc.scalar.copy(rd_bf, rd)
                            rd_d = dram.tile([1, 512], bf, tag="rd_d", bufs=3)
                            nc.gpsimd.dma_start(out=rd_d, in_=rd_bf)
                            rdb = prdb.tile([128, 512], bf, tag="rdb", bufs=2)
                            nc.sync.dma_start(
                                out=rdb, in_=rd_d.to_broadcast((128, 512))
                            )
                            nc.vector.tensor_mul(attn_i[:, h, :], pso, rdb)
                        attn_tiles.append(attn_i)

                    def oproj_chunk(i):
                        attn_i = attn_tiles[i]
                        for m in range(TT):
                            psf = psumD.tile([128, 512], f32, tag="psm", bufs=2)
                            for kk in range(HPC):
                                nc.tensor.matmul(
                                    out=psf,
                                    lhsT=ow[:, kk, ts(m, 128)],
                                    rhs=attn_i[:, kk, :],
                                    start=(kk == 0),
                                    stop=(kk == HPC - 1),
                                )
                            ob = pob.tile([128, 512], bf, tag="ob", bufs=3)
                            copy(eng(m), ob, psf)
                            nc.sync.dma_start(
                                out=out_ap[ts(m, 128), ts(i, 512)], in_=ob
                            )

                    attention_chunk(0)
                    for i in range(1, TQ):
                        attention_chunk(i)
                        oproj_chunk(i - 1)
                    oproj_chunk(TQ - 1)

    nc.compile()
    return nc


def _tile_w(w):
    """[K, N] -> [N/128, 128, K/128, 128] so each col-block loads contiguously."""
    K, N = w.shape
    return np.ascontiguousarray(
        w.reshape(K // 128, 128, N // 128, 128).transpose(2, 1, 0, 3))


def _prep(inputs):
    x = np.asarray(inputs["hidden_states"], np.float32)
    qaw = np.asarray(inputs["q_a_w"], np.float32)
    qalw = np.asarray(inputs["q_a_ln_w"], np.float32)
    qbw = np.asarray(inputs["q_b_w"], np.float32)
    kvaw = np.asarray(inputs["kv_a_w"], np.float32)
    kvlw = np.asarray(inputs["kv_a_ln_w"], np.float32)
    kvbw = np.asarray(inputs["kv_b_w"], np.float32)
    ow = np.asarray(inputs["o_w"], np.float32)

    scale = 1.0 / np.sqrt(np.float32(HEAD_DIM))
    qbw_f = (qbw * qalw[:, None] * scale).astype(BF16)
    kvbw_f = (kvbw * kvlw[:, None]).astype(BF16)
    qaw_t = _tile_w(qaw.astype(BF16))               # [NQ, 128, KC, 128]

    r = np.arange(128)[:, None]
    j = np.arange(896)[None, :]
    mask = np.where((j - 384) >= r, 1.0, 0.0).astype(BF16)
    ones128 = np.ones((128, 1), BF16)

    def lat_tiled(w):  # [KV_LORA, N] -> [128, NL, N] (p, kk, n)
        return np.ascontiguousarray(w.reshape(NL, 128, -1).transpose(1, 0, 2))

    in_maps = []
    for c in range(NCORES):
        b, g = c // 4, c % 4
        qbw_g = qbw_f[:, g * HPC * HEAD_DIM:(g + 1) * HPC * HEAD_DIM]
        pairs = []
        for pair in range(HPC // 2):
            h0, h1 = 2 * pair, 2 * pair + 1
            cols = np.concatenate([
                qbw_g[:, h0 * HEAD_DIM:h0 * HEAD_DIM + QK_NOPE],
                qbw_g[:, h1 * HEAD_DIM:h1 * HEAD_DIM + QK_NOPE],
                qbw_g[:, h0 * HEAD_DIM + QK_NOPE:(h0 + 1) * HEAD_DIM],
                qbw_g[:, h1 * HEAD_DIM + QK_NOPE:(h1 + 1) * HEAD_DIM],
            ], axis=1)  # [Q_LORA, 384]
            pairs.append(cols.reshape(NQ, 128, 384).transpose(1, 0, 2))
        qbw_c = np.ascontiguousarray(np.stack(pairs))   # [2, 128, NQ, 384]

        kvaw_g = np.concatenate(
            [kvaw[:, :KV_LORA],
             kvaw[:, KV_LORA + g * HPC * QK_ROPE:
                  KV_LORA + (g + 1) * HPC * QK_ROPE]], axis=1).astype(BF16)

        in_maps.append({
            "x": x[b].astype(BF16),
            "qaw": qaw_t,
            "kvaw": _tile_w(kvaw_g),
            "qbw": qbw_c,
            "kvbw": lat_tiled(kvbw_f[:, g * HPC * 256:(g + 1) * HPC * 256]),
            "ow": np.ascontiguousarray(
                ow[g * HPC * V_HEAD:(g + 1) * HPC * V_HEAD]
                .astype(BF16).reshape(HPC, 128, HIDDEN).transpose(1, 0, 2)),
            "mask": mask,
            "ones128": ones128,
        })
    return in_maps


def kernel(**inputs):
    from concourse.bass_utils import run_bass_kernel_spmd

    if "nc" not in _CACHE:
        _CACHE["nc"] = _build()
    nc = _CACHE["nc"]
    in_maps = _prep(inputs)
    res = run_bass_kernel_spmd(nc, in_maps, core_ids=list(range(NCORES)),
                               **_CACHE.get("run_kwargs", {}))
    _CACHE["last_results"] = res
    out = np.zeros((B, T, HIDDEN), np.float32)
    for c in range(NCORES):
        out[c // 4] += np.asarray(res.results[c]["out"], np.float32).T
    return out
